# revision 15
# baseline (speedup 1.0000x reference)
"""Trainium2 Bass kernel for the KernelAttention module.

Sharding: the 4096 query positions (H*W) are split into 8 contiguous
blocks of 512, one per NeuronCore. The softmax mixes only across
(camera, g) at a FIXED query position, so this split needs no
collectives: every core computes its 512 output rows end-to-end.

Device-side layout strategy (per core):
  - activations live as [128 rows, 256 feat] tiles (rows on partitions)
  - LayerNorm stats via bn_stats/bn_aggr (free-dim reduction, native)
  - normalized tiles are transposed on the TensorEngine (2x 128x128)
    to produce the lhsT operand for B1-orientation matmuls:
        psum[rows, dout] += xT[k-tile].T @ W[k-tile]
    with float32r (full-rate fp32 matmul mode, moving dim >= 256)
  - LN gain and the attention 1/sqrt(dh) scale are folded into the
    projection weights on the host
  - scores/softmax/attn*v are computed with DVE/ACT elementwise ops in
    the rows-on-partitions layout; the mask is pre-broadcast on host
"""

import os

import numpy as np
from contextlib import ExitStack

import concourse.bass as bass
import concourse.mybir as mybir
import concourse.tile as tile
from concourse import bacc
from concourse.bass import ts
from concourse.bass_utils import run_bass_kernel_spmd
from concourse.masks import make_identity

P = 128
N_CAM, G, HEADS, DH, D = 6, 8, 4, 64, 256
NCORES = 8
QLEN = 4096
S = QLEN // NCORES          # 512 positions per core
NST = S // P                # 4 s-tiles per core
NG = N_CAM * G              # 48
FREE_SC = HEADS * NG        # 192
EPS = 1e-5
SCALE = DH ** -0.5
F32 = mybir.dt.float32
F32R = mybir.dt.float32r
AX = mybir.AxisListType
ALU = mybir.AluOpType
ACTF = mybir.ActivationFunctionType

_PROGRAM_CACHE = {}


def _build_program():
    nc = bacc.Bacc(
        "TRN2",
        target_bir_lowering=False,
        debug=False,
        enable_asserts=False,
        num_devices=NCORES,
    )

    qx_d = nc.dram_tensor("qx", (N_CAM * S, D), F32, kind="ExternalInput")
    kx_d = nc.dram_tensor("kx", (NG * S, D), F32, kind="ExternalInput")
    vx_d = nc.dram_tensor("vx", (NG * S, D), F32, kind="ExternalInput")
    am_d = nc.dram_tensor("amask", (S, FREE_SC), F32, kind="ExternalInput")
    sk_d = nc.dram_tensor("skipx", (S, D), F32, kind="ExternalInput")
    wq_d = nc.dram_tensor("wq", (2, P, D), F32, kind="ExternalInput")
    wk_d = nc.dram_tensor("wk", (2, P, D), F32, kind="ExternalInput")
    wv_d = nc.dram_tensor("wv", (2, P, D), F32, kind="ExternalInput")
    wp_d = nc.dram_tensor("wp", (2, P, D), F32, kind="ExternalInput")
    w1_d = nc.dram_tensor("w1", (2, P, 2 * D), F32, kind="ExternalInput")
    w2_d = nc.dram_tensor("w2", (4, P, D), F32, kind="ExternalInput")
    out_d = nc.dram_tensor("out", (S, D), F32, kind="ExternalOutput")

    with tile.TileContext(nc) as tc, ExitStack() as ctx:
        const = ctx.enter_context(tc.tile_pool(name="const", bufs=1))
        xin_p = ctx.enter_context(tc.tile_pool(name="xin", bufs=10))
        st_p = ctx.enter_context(tc.tile_pool(name="stats", bufs=24))
        xn_p = ctx.enter_context(tc.tile_pool(name="xn", bufs=8))
        xt_p = ctx.enter_context(tc.tile_pool(name="xt", bufs=8))
        pr_p = ctx.enter_context(tc.tile_pool(name="pr", bufs=8))
        pt_p = ctx.enter_context(tc.tile_pool(name="ptr", bufs=3, space="PSUM"))
        pm_p = ctx.enter_context(tc.tile_pool(name="pmm", bufs=3, space="PSUM"))
        pl_p = ctx.enter_context(tc.tile_pool(name="pmlp", bufs=2, space="PSUM"))
        qp_p = ctx.enter_context(tc.tile_pool(name="qp", bufs=N_CAM * NST))
        kp_p = ctx.enter_context(tc.tile_pool(name="kp", bufs=4))
        vp_p = ctx.enter_context(tc.tile_pool(name="vp", bufs=56))
        sc_p = ctx.enter_context(tc.tile_pool(name="sc", bufs=3))
        sm_p = ctx.enter_context(tc.tile_pool(name="sm", bufs=4))
        ac_p = ctx.enter_context(tc.tile_pool(name="acc", bufs=2))
        po_p = ctx.enter_context(tc.tile_pool(name="post", bufs=2))

        ident_f = const.tile([P, P], F32, tag="ident_f")
        make_identity(nc, ident_f[:])
        ident = const.tile([P, P], F32R, tag="ident")
        nc.any.tensor_copy(ident[:], ident_f[:])
        identr = ident[:]
        eps_t = const.tile([P, 1], F32, tag="eps")
        nc.any.memset(eps_t[:], EPS)

        def load_w(d, kt, nn, name):
            stg = const.tile([P, kt, nn], F32, tag="wstg", name=f"stg_{name}")
            nc.sync.dma_start(stg[:], d.ap().rearrange("t p n -> p t n"))
            t = const.tile([P, kt, nn], F32R, tag=name)
            nc.any.tensor_copy(t[:], stg[:])
            return t

        wq_t = load_w(wq_d, 2, D, "wq")
        wk_t = load_w(wk_d, 2, D, "wk")
        wv_t = load_w(wv_d, 2, D, "wv")
        wp_t = load_w(wp_d, 2, D, "wp")
        w1_t = load_w(w1_d, 2, 2 * D, "w1")
        w2_t = load_w(w2_d, 4, D, "w2")

        def ln_stats(x):
            """Returns agg tile; [:,3:4]=rstd, [:,2:3]=-mean*rstd."""
            bns = st_p.tile([P, 6], F32, tag="bns")
            nc.vector.bn_stats(bns[:], x[:])
            agg = st_p.tile([P, 4], F32, tag="agg")
            nc.vector.bn_aggr(agg[:, 0:2], bns[:])
            nc.scalar.activation(agg[:, 2:3], agg[:, 1:2], ACTF.Sqrt, bias=eps_t[:])
            nc.vector.reciprocal(agg[:, 3:4], agg[:, 2:3])
            nc.vector.tensor_scalar(
                agg[:, 2:3], agg[:, 0:1], agg[:, 3:4], -1.0,
                op0=ALU.mult, op1=ALU.mult,
            )
            return agg

        def ln_normalize(x, out_pool, tag):
            agg = ln_stats(x)
            xn = out_pool.tile([P, D], F32R, tag=tag)
            nc.any.tensor_scalar(
                xn[:], x[:], agg[:, 3:4], agg[:, 2:3],
                op0=ALU.mult, op1=ALU.add,
            )
            return xn

        def transpose_to_sbuf(xn, nk):
            """[P, nk*128] rows-major tile -> [P, nk*128] transposed tile."""
            pt = pt_p.tile([P, nk * P], F32, tag="pt")
            for t in range(nk):
                nc.tensor.transpose(
                    pt[:, ts(t, P)].bitcast(F32R),
                    xn[:, ts(t, P)].bitcast(F32R),
                    identr,
                )
            xt = xt_p.tile([P, nk * P], F32R, tag="xt")
            nc.any.tensor_copy(xt[:], pt[:])
            return xt

        def proj_matmul(xt, w_t, nk, nn, psum_pool):
            ps = psum_pool.tile([P, nn], F32, tag=f"ps{nn}")
            for t in range(nk):
                nc.tensor.matmul(
                    ps[:],
                    lhsT=xt[:, ts(t, P)],
                    rhs=w_t[:, t, :],
                    start=(t == 0),
                    stop=(t == nk - 1),
                )
            return ps

        def ln_proj(src_ap, w_t, out_pool, tag):
            """DMA row-tile, LN (no gain/bias: folded in W), project."""
            x = xin_p.tile([P, D], F32, tag="xin")
            nc.sync.dma_start(x[:], src_ap)
            xn = ln_normalize(x, xn_p, "xn")
            xt = transpose_to_sbuf(xn, 2)
            ps = proj_matmul(xt, w_t, 2, D, pm_p)
            out = out_pool.tile([P, D], F32, tag=tag)
            nc.any.tensor_copy(out[:], ps[:])
            return out

        # ---- Phase Q: 24 projected q tiles, resident ----
        qp_tiles = {}
        for n in range(N_CAM):
            for st in range(NST):
                row0 = n * S + st * P
                qp_tiles[(n, st)] = ln_proj(
                    qx_d.ap()[row0:row0 + P, :], wq_t, qp_p, "qp"
                )

        # ---- Main: per s-tile ----
        for st in range(NST):
            sc = sc_p.tile([P, HEADS, N_CAM, G], F32, tag="sc")
            vp_tiles = {}
            for n in range(N_CAM):
                qpt = qp_tiles[(n, st)]
                for g in range(G):
                    blk = (n * G + g) * S + st * P
                    kp = ln_proj(kx_d.ap()[blk:blk + P, :], wk_t, kp_p, "kp")
                    vp = ln_proj(vx_d.ap()[blk:blk + P, :], wv_t, vp_p, "vp")
                    vp_tiles[(n, g)] = vp
                    prod = pr_p.tile([P, D], F32, tag="prod")
                    nc.gpsimd.tensor_tensor(prod[:], kp[:], qpt[:], op=ALU.mult)
                    nc.vector.tensor_reduce(
                        sc[:, :, n, g],
                        prod[:].rearrange("p (m d) -> p m d", m=HEADS),
                        op=ALU.add,
                        axis=AX.X,
                    )

            # mask + softmax over (n, g) per head
            am = xin_p.tile([P, HEADS, N_CAM, G], F32, tag="am")
            nc.sync.dma_start(am[:], am_d.ap()[ts(st, P), :])
            nc.gpsimd.tensor_tensor(sc[:], sc[:], am[:], op=ALU.add)
            nm = sm_p.tile([P, HEADS], F32, tag="nm")
            nc.vector.tensor_reduce(
                nm[:],
                sc[:].rearrange("p m n g -> p m (n g)"),
                op=ALU.max,
                axis=AX.X,
                negate=True,
            )
            att = sc_p.tile([P, HEADS, N_CAM, G], F32, tag="att")
            se = sm_p.tile([P, HEADS], F32, tag="se")
            for m in range(HEADS):
                nc.scalar.activation(
                    att[:, m], sc[:, m], ACTF.Exp,
                    bias=nm[:, m:m + 1], accum_out=se[:, m:m + 1],
                )
            rc = sm_p.tile([P, HEADS], F32, tag="rc")
            nc.vector.reciprocal(rc[:], se[:])
            for m in range(HEADS):
                nc.any.tensor_scalar_mul(att[:, m], att[:, m], rc[:, m:m + 1])

            # attn @ v : 4 parallel accumulation chains
            accs = [
                ac_p.tile([P, D], F32, tag=f"acc{j}", name=f"acc{j}_{st}")
                for j in range(4)
            ]
            idx = 0
            for n in range(N_CAM):
                for g in range(G):
                    vp = vp_tiles.pop((n, g))
                    j, r = divmod(idx, 12)
                    attb = att[:, :, n, g][:, :, None].broadcast_to(
                        (P, HEADS, DH)
                    )
                    vpv = vp[:].rearrange("p (m d) -> p m d", m=HEADS)
                    accv = accs[j][:].rearrange("p (m d) -> p m d", m=HEADS)
                    if r == 0:
                        nc.any.tensor_tensor(accv, attb, vpv, op=ALU.mult)
                    else:
                        prod2 = pr_p.tile([P, D], F32, tag="prod2")
                        p2v = prod2[:].rearrange("p (m d) -> p m d", m=HEADS)
                        nc.any.tensor_tensor(p2v, attb, vpv, op=ALU.mult)
                        eng = nc.gpsimd if (j % 2 == 0) else nc.any
                        eng.tensor_tensor(
                            accs[j][:], accs[j][:], prod2[:], op=ALU.add
                        )
                    idx += 1
            nc.any.tensor_tensor(accs[0][:], accs[0][:], accs[1][:], op=ALU.add)
            nc.any.tensor_tensor(accs[2][:], accs[2][:], accs[3][:], op=ALU.add)
            a_t = ac_p.tile([P, D], F32R, tag="a")
            nc.any.tensor_tensor(a_t[:], accs[0][:], accs[2][:], op=ALU.add)

            # ---- post-attention: proj + skip, ln_pre, mlp, ln_post ----
            at = transpose_to_sbuf(a_t, 2)
            ps = proj_matmul(at, wp_t, 2, D, pm_p)
            sk = xin_p.tile([P, D], F32, tag="sk")
            nc.sync.dma_start(sk[:], sk_d.ap()[ts(st, P), :])
            z = po_p.tile([P, D], F32, tag="z")
            nc.any.tensor_tensor(z[:], ps[:], sk[:], op=ALU.add)

            zn = ln_normalize(z, po_p, "zn")

            znt = transpose_to_sbuf(zn, 2)
            ps1 = proj_matmul(znt, w1_t, 2, 2 * D, pl_p)
            h1 = po_p.tile([P, 2 * D], F32R, tag="h1")
            nc.scalar.activation(h1[:], ps1[:], ACTF.Gelu)

            h1t = transpose_to_sbuf(h1, 4)
            ps2 = proj_matmul(h1t, w2_t, 4, D, pm_p)
            z2 = po_p.tile([P, D], F32, tag="z2")
            nc.any.tensor_tensor(z2[:], ps2[:], zn[:].bitcast(F32), op=ALU.add)

            zo = ln_normalize(z2, po_p, "zo")
            nc.sync.dma_start(out_d.ap()[ts(st, P), :], zo[:].bitcast(F32))

    if not os.environ.get("KERNEL_SKIP_COMPILE"):
        nc.compile()
    return nc


def _get_program():
    if "p" not in _PROGRAM_CACHE:
        _PROGRAM_CACHE["p"] = _build_program()
    return _PROGRAM_CACHE["p"]


def kernel(q, k, v, skip, mask,
           ln_q_g, ln_q_b, wq, bq,
           ln_k_g, ln_k_b, wk, bk,
           ln_v_g, ln_v_b, wv, bv,
           w_proj, b_proj,
           ln_pre_g, ln_pre_b,
           w_mlp1, b_mlp1, w_mlp2, b_mlp2,
           ln_post_g, ln_post_b):
    q = np.asarray(q, np.float32)
    k = np.asarray(k, np.float32)
    v = np.asarray(v, np.float32)
    skip = np.asarray(skip, np.float32)
    mask = np.asarray(mask)

    # fold LN gains (and attention scale for q) into projection weights;
    # the corresponding biases are all zero in this model instance --
    # assert rather than silently drop them.
    f = np.float32
    wqf = (np.asarray(ln_q_g)[:, None] * np.asarray(wq) * SCALE).astype(f)
    wkf = (np.asarray(ln_k_g)[:, None] * np.asarray(wk)).astype(f)
    wvf = (np.asarray(ln_v_g)[:, None] * np.asarray(wv)).astype(f)
    for name, val in [
        ("bq'", np.asarray(ln_q_b) @ np.asarray(wq) + np.asarray(bq)),
        ("bk'", np.asarray(ln_k_b) @ np.asarray(wk) + np.asarray(bk)),
        ("bv'", np.asarray(ln_v_b) @ np.asarray(wv) + np.asarray(bv)),
        ("b_proj", np.asarray(b_proj)),
        ("b_mlp1", np.asarray(b_mlp1)),
        ("b_mlp2", np.asarray(b_mlp2)),
        ("ln_pre_b", np.asarray(ln_pre_b)),
        ("ln_post_b", np.asarray(ln_post_b)),
    ]:
        assert np.allclose(val, 0.0, atol=1e-12), f"{name} nonzero: unsupported"
    for name, val in [("ln_pre_g", ln_pre_g), ("ln_post_g", ln_post_g)]:
        assert np.allclose(np.asarray(val), 1.0), f"{name} != 1: unsupported"

    wpf = np.ascontiguousarray(np.asarray(w_proj, f))
    w1f = np.ascontiguousarray(np.asarray(w_mlp1, f))
    w2f = np.ascontiguousarray(np.asarray(w_mlp2, f))

    wq_p = np.ascontiguousarray(wqf.reshape(2, P, D))
    wk_p = np.ascontiguousarray(wkf.reshape(2, P, D))
    wv_p = np.ascontiguousarray(wvf.reshape(2, P, D))
    wp_p = np.ascontiguousarray(wpf.reshape(2, P, D))
    w1_p = np.ascontiguousarray(w1f.reshape(2, P, 2 * D))
    w2_p = np.ascontiguousarray(w2f.reshape(4, P, D))

    # host-side data layout prep
    qx_all = np.ascontiguousarray(
        q[0].transpose(0, 2, 3, 1).reshape(N_CAM, QLEN, D)
    )
    skip_all = np.ascontiguousarray(
        skip[0].transpose(1, 2, 0).reshape(QLEN, D)
    )
    mask_all = mask[0, :, :, 0].astype(bool)  # (6, 4096)

    in_maps = []
    for c in range(NCORES):
        sl = slice(c * S, (c + 1) * S)
        qx_c = np.ascontiguousarray(qx_all[:, sl, :]).reshape(N_CAM * S, D)
        kx_c = np.ascontiguousarray(
            k[0][:, sl].transpose(0, 2, 1, 3)
        ).reshape(NG * S, D)
        vx_c = np.ascontiguousarray(
            v[0][:, sl].transpose(0, 2, 1, 3)
        ).reshape(NG * S, D)
        mc = mask_all[:, sl]                       # (6, 512)
        amc = np.where(mc.T, f(0.0), f(-1e9)).astype(f)  # (512, 6)
        am_c = np.ascontiguousarray(
            np.broadcast_to(amc[:, None, :, None], (S, HEADS, N_CAM, G))
        ).reshape(S, FREE_SC)
        in_maps.append({
            "qx": qx_c, "kx": kx_c, "vx": vx_c,
            "amask": am_c,
            "skipx": np.ascontiguousarray(skip_all[sl]),
            "wq": wq_p, "wk": wk_p, "wv": wv_p, "wp": wp_p,
            "w1": w1_p, "w2": w2_p,
        })

    global _LAST_IN_MAPS
    _LAST_IN_MAPS = in_maps
    nc = _get_program()
    res = run_bass_kernel_spmd(nc, in_maps, core_ids=list(range(NCORES)))
    z = np.concatenate([res.results[c]["out"] for c in range(NCORES)], axis=0)
    out = z.reshape(64, 64, D).transpose(2, 0, 1)[None]
    return np.ascontiguousarray(out.astype(np.float32))
